# revision 1
# baseline (speedup 1.0000x reference)
"""Bigram LM loss kernel for 8 Trainium2 NeuronCores.

reference:
    x = emb[idx]                         # [B,S,D] gather
    logits = einsum('bsd,vd->bsv', x, W) + b
    loss = -mean(log_softmax(logits)[targets])
    returns (logits, loss)

Strategy (vocab/tensor parallel, per the lm_head sharding):
  - Pad V=50257 -> 50264 = 8*6283; core k owns vocab slice [k*6283,(k+1)*6283).
  - W is pre-transposed + bf16-cast on host -> each core keeps its W_T shard
    [1024, 6283] resident in SBUF; matmul runs in bf16 with f32 PSUM accum.
  - emb is bf16-cast on host; x rows are gathered on-device by indirect DMA
    (128 rows/tile) and transposed to [d, t] layout with the DMA xbar.
  - Per 128-token tile: 13 vocab tiles x 8 K-tiles of matmul; DVE adds the
    (partition-broadcast) bias while draining PSUM->SBUF; ACT computes
    exp(logits) with fused per-token row-sum (accum_out) for the softmax
    denominator partials.
  - Host gathers the 8 logits shards, combines per-token sum-exp partials
    (the "all-reduce" on the per-token logsumexp), extracts target logits,
    and forms the scalar loss.
"""

import numpy as np
import ml_dtypes

import concourse.bass as bass
import concourse.mybir as mybir
import concourse.tile as tile
from concourse import bacc
from concourse.bass_utils import run_bass_kernel_spmd

# Problem constants (hardcoded per contract)
V = 50257
D = 1024
B, S = 4, 2048
T = B * S  # 8192 tokens
NCORES = 8
VS = 6283  # per-core vocab shard (V padded to 50264)
VP = VS * NCORES
P = 128
TT = T // P  # 64 token tiles
KT = D // P  # 8 contraction tiles
NV = 512  # vocab tile (one PSUM bank of f32)
VTILES = [(j * NV, min(NV, VS - j * NV)) for j in range((VS + NV - 1) // NV)]
PAD_BIAS = -30000.0  # exp() underflows to 0; padded cols are discarded on host

BF16 = mybir.dt.bfloat16
F32 = mybir.dt.float32

LAST_RESULT = None  # stashed BassKernelResults for the test harness

_CACHED_NC = None


def _build_nc():
    nc = bacc.Bacc("TRN2", target_bir_lowering=False, debug=False, num_devices=NCORES)

    wt = nc.dram_tensor("wt", [D, VS], BF16, kind="ExternalInput").ap()
    emb = nc.dram_tensor("emb", [V, D], BF16, kind="ExternalInput").ap()
    idxt = nc.dram_tensor("idxt", [P, TT], mybir.dt.int32, kind="ExternalInput").ap()
    bias = nc.dram_tensor("bias", [VS], F32, kind="ExternalInput").ap()

    logits = nc.dram_tensor("logits", [T, VS], F32, kind="ExternalOutput").ap()
    sume = nc.dram_tensor("sume", [P, TT], F32, kind="ExternalOutput").ap()

    with tile.TileContext(nc) as tc:
        with (
            tc.tile_pool(name="persist", bufs=1) as persist,
            tc.tile_pool(name="xgp", bufs=2) as xgp,
            tc.tile_pool(name="xtp", bufs=2) as xtp,
            tc.tile_pool(name="psp", bufs=8, space="PSUM") as psp,
            tc.tile_pool(name="lrp", bufs=2) as lrp,
            tc.tile_pool(name="etp", bufs=2) as etp,
            tc.tile_pool(name="secp", bufs=2) as secp,
        ):
            # --- one-time loads ---
            wsb = persist.tile([P, KT, VS], BF16)
            for k in range(KT):
                nc.sync.dma_start(wsb[:, k, :], wt[k * P : (k + 1) * P, :])
            # bias broadcast across partitions (DRAM step-0 partition reads)
            bb = persist.tile([P, VS], F32)
            nc.sync.dma_start(bb[:], bass.AP(bias.tensor, 0, [[0, P], [1, VS]]))
            idx_sb = persist.tile([P, TT], mybir.dt.int32)
            nc.sync.dma_start(idx_sb[:], idxt[:])
            sume_sb = persist.tile([P, TT], F32)

            for i in range(TT):
                # gather this tile's 128 embedding rows: xg[p, :] = emb[idx[p], :]
                xg = xgp.tile([P, D], BF16, tag="xg")
                nc.gpsimd.indirect_dma_start(
                    out=xg[:],
                    out_offset=None,
                    in_=emb[:],
                    in_offset=bass.IndirectOffsetOnAxis(
                        ap=idx_sb[:, i : i + 1], axis=0
                    ),
                )
                # xbar transpose to lhsT layout: xt[p, k, t] = xg[t, k*128+p]
                xt = xtp.tile([P, KT, P], BF16, tag="xt")
                nc.scalar.dma_start(xt[:], xg[:], transpose=True)

                lrow = lrp.tile([P, VS], F32, tag="lrow")
                sec = secp.tile([P, len(VTILES)], F32, tag="sec")
                for j, (v0, n) in enumerate(VTILES):
                    ps = psp.tile([P, NV], F32, tag="ps")
                    for k in range(KT):
                        nc.tensor.matmul(
                            ps[:, :n],
                            xt[:, k, :],
                            wsb[:, k, v0 : v0 + n],
                            start=(k == 0),
                            stop=(k == KT - 1),
                        )
                    # drain PSUM -> SBUF with fused bias add
                    nc.vector.tensor_add(
                        lrow[:, v0 : v0 + n], ps[:, :n], bb[:, v0 : v0 + n]
                    )
                    # exp with fused per-token row-sum for softmax denominator
                    et = etp.tile([P, NV], F32, tag="et")
                    nc.scalar.activation(
                        et[:, :n],
                        lrow[:, v0 : v0 + n],
                        mybir.ActivationFunctionType.Exp,
                        accum_out=sec[:, j : j + 1],
                    )
                nc.sync.dma_start(logits[i * P : (i + 1) * P, :], lrow[:])
                nc.vector.tensor_reduce(
                    sume_sb[:, i : i + 1],
                    sec[:],
                    axis=mybir.AxisListType.X,
                    op=mybir.AluOpType.add,
                )
            nc.sync.dma_start(sume[:], sume_sb[:])

    nc.compile()
    return nc


def kernel(idx, targets, emb, W, b):
    global LAST_RESULT, _CACHED_NC

    idx = np.asarray(idx).astype(np.int32).reshape(T)
    targets = np.asarray(targets).astype(np.int64).reshape(T)
    emb = np.asarray(emb, dtype=np.float32)
    W = np.asarray(W, dtype=np.float32)
    b = np.asarray(b, dtype=np.float32)

    # ---- host-side input prep (sharding + layout) ----
    emb_bf = emb.astype(ml_dtypes.bfloat16)
    wt_full = np.zeros((D, VP), dtype=ml_dtypes.bfloat16)
    wt_full[:, :V] = np.ascontiguousarray(W.T).astype(ml_dtypes.bfloat16)
    b_pad = np.full(VP, PAD_BIAS, dtype=np.float32)
    b_pad[:V] = b
    idxt = np.ascontiguousarray(idx.reshape(TT, P).T)  # [P, TT] int32

    in_maps = []
    for k in range(NCORES):
        sl = slice(k * VS, (k + 1) * VS)
        in_maps.append(
            {
                "wt": np.ascontiguousarray(wt_full[:, sl]),
                "emb": emb_bf,
                "idxt": idxt,
                "bias": np.ascontiguousarray(b_pad[sl]),
            }
        )

    if _CACHED_NC is None:
        _CACHED_NC = _build_nc()
    nc = _CACHED_NC

    res = run_bass_kernel_spmd(nc, in_maps, core_ids=list(range(NCORES)))
    LAST_RESULT = res

    # ---- host-side unshard / combine ----
    logits_flat = np.empty((T, V), dtype=np.float32)
    sumexp = np.zeros(T, dtype=np.float64)
    for k in range(NCORES):
        r = res.results[k]
        v0 = k * VS
        w = min(VS, V - v0)
        logits_flat[:, v0 : v0 + w] = r["logits"][:, :w]
        # sume[p, i] is token i*128+p
        sumexp += r["sume"].T.reshape(T).astype(np.float64)

    logsumexp = np.log(sumexp)  # [T]
    tgt_logit = logits_flat[np.arange(T), targets].astype(np.float64)
    loss = np.float32(np.mean(logsumexp - tgt_logit))

    logits_out = logits_flat.reshape(B, S, V)
    return logits_out, loss
